# revision 13
# baseline (speedup 1.0000x reference)
"""Trainium2 Bass kernel for PointPillarClusterFusion.

Key restructuring (validated against the reference in numpy):
  * The whole per-point MLP depends only on the point's voxel (b, gy, gx),
    so it is computed once per grid cell (grid MLP, BN folded into W1/S1),
    then per-point values are an irregular gather of grid rows.
  * Points are sorted by cluster id (= rank of cinds among sorted unique
    values); cluster segment-sums become small dense matmuls against 0/1
    "staircase" matrices built on-device with an is_equal DVE op.
  * Sharding: 8 contiguous cid_x ranges; each core gets a bev x-slab and
    all points/clusters of its range, so segment reductions stay local.

Per core (identical program, different data):
  A) grid MLP over its slab (384->256->128->64->1), write per-cell rows
     [feat128 | prob | pad] to a DRAM scratch (grid_T), logits to an output.
  B) per cluster-page (128 clusters): dma_gather of its points' grid rows,
     staircase matmuls accumulate feat/xyz/prob/count sums in PSUM, divide
     by counts via per-page PE transposes; gathered feat rows DMA out as
     the point_feats output (sorted order; host inverse-permutes).
  C) cluster MLP (128->256->768) on the cluster means, transpose, DMA out.
"""

import os
import sys
import numpy as np

# --- problem constants (from the reference) ---
VOX = np.float32(0.4)
X0 = np.float32(-51.2)
CLUSTER = np.float32(2.0)
MAXC = 8192
B, C, H, W = 2, 384, 256, 256
N = 200000
BN_EPS = np.float32(1e-5)

# --- sharding / layout constants ---
NCORES = 8
BOUNDS = [0, 7, 13, 20, 26, 33, 39, 46, 52]  # cid_x range per core
W_SLAB = 38
CELLS_B = H * W_SLAB            # 9728 cells per batch in the slab
CELLS = B * CELLS_B             # 19456
TILE = 512
NT_B = CELLS_B // TILE          # 19 tiles per batch
PAGE = 128                      # clusters per page
NPAGES = 6
CMAX = PAGE * NPAGES            # 768 cluster slots per core
GROW = 192                      # grid_T row floats (768 B, 256-multiple)

f32 = np.float32

TRACE = False           # set by test harness to capture an NTFF profile
LAST_RESULTS = None     # BassKernelResults of the last kernel() call


def _install_ntff_shim():
    """bass_utils imports antenv.axon_hooks when tracing; provide it via
    ctypes against libaxon_pjrt.so if the real module is absent."""
    try:
        import antenv.axon_hooks  # noqa: F401
        return
    except ImportError:
        pass
    import contextlib
    import ctypes
    import types

    so_path = "/opt/axon/libaxon_pjrt.so"
    hook = None
    try:
        lib = ctypes.CDLL(so_path)
        if hasattr(lib, "axon_start_nrt_profile"):
            lib.axon_start_nrt_profile.argtypes = [
                ctypes.POINTER(ctypes.c_int64),
                ctypes.c_size_t,
            ]
            lib.axon_start_nrt_profile.restype = ctypes.c_int64
            lib.axon_stop_nrt_profile.argtypes = [ctypes.c_char_p]
            lib.axon_stop_nrt_profile.restype = ctypes.c_int64

            @contextlib.contextmanager
            def _profile(output_dir, device_ids):
                import jax
                jax.devices()
                if device_ids:
                    ids = (ctypes.c_int64 * len(device_ids))(*device_ids)
                    rc = lib.axon_start_nrt_profile(ids, len(device_ids))
                else:
                    rc = lib.axon_start_nrt_profile(None, 0)
                if rc != 0:
                    raise RuntimeError(f"axon_start_nrt_profile rc={rc}")
                try:
                    yield
                finally:
                    n = lib.axon_stop_nrt_profile(str(output_dir).encode())
                    print(f"ntff profile: {n} file(s) -> {output_dir}",
                          file=sys.stderr)

            hook = _profile
    except OSError:
        pass

    mod = types.ModuleType("antenv.axon_hooks")
    mod.get_axon_ntff_profile_hook = lambda: hook
    mod.set_axon_ntff_profile_hook = lambda h: None
    try:
        import antenv
        antenv.axon_hooks = mod
    except ImportError:
        pass
    sys.modules["antenv.axon_hooks"] = mod


PHASES = "abc"  # bisection knob: which phases to emit


def _build_program(nblk_page):
    import concourse.bacc as bacc
    import concourse.mybir as mybir
    import concourse.tile as tile

    dt = mybir.dt.float32
    AF = mybir.ActivationFunctionType
    ALU = mybir.AluOpType
    nblk = NPAGES * nblk_page

    nc = bacc.Bacc("TRN2", target_bir_lowering=False, debug=False,
                   num_devices=NCORES)

    # ---- external tensors ----
    bev_in = nc.dram_tensor("bev_in", [B, 3, 128, CELLS_B], dt,
                            kind="ExternalInput")
    gidx_in = nc.dram_tensor("gidx_in", [128, nblk * 8], mybir.dt.int16,
                             kind="ExternalInput")
    cval_in = nc.dram_tensor("cval_in", [128, nblk], dt, kind="ExternalInput")
    xyzp_in = nc.dram_tensor("xyzp_in", [128, nblk, 5], dt,
                             kind="ExternalInput")
    iota_in = nc.dram_tensor("iota_in", [128, PAGE], dt, kind="ExternalInput")
    ident_in = nc.dram_tensor("ident_in", [128, 128], dt,
                              kind="ExternalInput")
    w1_in = nc.dram_tensor("w1_in", [128, 3, 256], dt, kind="ExternalInput")
    b1_in = nc.dram_tensor("b1_in", [128, 2], dt, kind="ExternalInput")
    w2_in = nc.dram_tensor("w2_in", [128, 2, 128], dt, kind="ExternalInput")
    b2_in = nc.dram_tensor("b2_in", [128, 1], dt, kind="ExternalInput")
    s1_in = nc.dram_tensor("s1_in", [128, 128], dt, kind="ExternalInput")
    sb1_in = nc.dram_tensor("sb1_in", [128, 1], dt, kind="ExternalInput")
    s2_in = nc.dram_tensor("s2_in", [128, 1], dt, kind="ExternalInput")
    sb2_in = nc.dram_tensor("sb2_in", [128, 1], dt, kind="ExternalInput")
    p1_in = nc.dram_tensor("p1_in", [128, 256], dt, kind="ExternalInput")
    pb1_in = nc.dram_tensor("pb1_in", [128, 2], dt, kind="ExternalInput")
    p2_in = nc.dram_tensor("p2_in", [128, 2, 768], dt, kind="ExternalInput")
    pb2_in = nc.dram_tensor("pb2_in", [128, 6], dt, kind="ExternalInput")

    pf_out = nc.dram_tensor("pf_out", [nblk * 128, 128], dt,
                            kind="ExternalOutput")
    gl_out = nc.dram_tensor("gl_out", [128, CELLS // 128], dt,
                            kind="ExternalOutput")
    c4_out = nc.dram_tensor("c4_out", [CMAX, 4], dt, kind="ExternalOutput")
    cp_out = nc.dram_tensor("cp_out", [CMAX, 768], dt, kind="ExternalOutput")

    with tile.TileContext(nc) as tc:
        with (
            tc.tile_pool(name="const", bufs=1) as cpool,
            tc.tile_pool(name="work", bufs=3) as wpool,
            tc.tile_pool(name="stage", bufs=3) as spool,
            tc.tile_pool(name="gath", bufs=2) as gpool,
            tc.tile_pool(name="persist", bufs=1) as ppool,
            tc.tile_pool(name="ps512", bufs=4, space="PSUM") as ps512,
            tc.tile_pool(name="ps128", bufs=4, space="PSUM") as ps128,
            tc.tile_pool(name="dram", bufs=1, space="DRAM") as dpool,
        ):
            # ---- const preloads ----
            def load_const(name, ap, shape):
                t = cpool.tile(shape, ap.dtype, tag=name)
                nc.sync.dma_start(t[:], ap)
                return t

            w1_sb = load_const("w1", w1_in[:, :, :], [128, 3, 256])
            b1_sb = load_const("b1", b1_in[:, :], [128, 2])
            w2_sb = load_const("w2", w2_in[:, :, :], [128, 2, 128])
            b2_sb = load_const("b2", b2_in[:, :], [128, 1])
            s1_sb = load_const("s1", s1_in[:, :], [128, 128])
            sb1_sb = load_const("sb1", sb1_in[:, :], [128, 1])
            s2_sb = load_const("s2", s2_in[:, :], [128, 1])
            sb2_sb = load_const("sb2", sb2_in[:, :], [128, 1])
            p1_sb = load_const("p1", p1_in[:, :], [128, 256])
            pb1_sb = load_const("pb1", pb1_in[:, :], [128, 2])
            p2_sb = load_const("p2", p2_in[:, :, :], [128, 2, 768])
            pb2_sb = load_const("pb2", pb2_in[:, :], [128, 6])
            iota_sb = load_const("iota", iota_in[:, :], [128, PAGE])
            ident_sb = load_const("ident", ident_in[:, :], [128, 128])
            gidx_sb = load_const("gidx", gidx_in[:, :], [128, nblk * 8])
            cval_sb = load_const("cval", cval_in[:, :], [128, nblk])
            xyzp_sb = load_const("xyzp", xyzp_in[:, :, :], [128, nblk, 5])

            glog_sb = ppool.tile([128, CELLS // 128], dt, tag="glog")
            meanF = ppool.tile([128, CMAX], dt, tag="meanF")
            c4st = ppool.tile([128, NPAGES, 4], dt, tag="c4st")

            grid_T = dpool.tile([CELLS, GROW], dt, tag="gridT")

            # ================= phase A: grid MLP =================
            for b in range(B if "a" in PHASES else 0):
                for t in range(NT_B):
                    c0 = t * TILE
                    bevt = []
                    for ck in range(3):
                        bt = wpool.tile([128, TILE], dt, tag=f"bev{ck}")
                        nc.sync.dma_start(bt[:], bev_in[b, ck, :, c0:c0 + TILE])
                        bevt.append(bt)
                    # L1: 384 -> 256 (two M chunks), folded BN + relu
                    v1 = []
                    for mc in range(2):
                        ps = ps512.tile([128, TILE], dt, tag="b512")
                        for ck in range(3):
                            nc.tensor.matmul(
                                ps[:], w1_sb[:, ck, mc * 128:(mc + 1) * 128],
                                bevt[ck][:], start=(ck == 0), stop=(ck == 2))
                        vt = wpool.tile([128, TILE], dt, tag=f"v1{mc}")
                        nc.scalar.activation(vt[:], ps[:], AF.Relu,
                                             bias=b1_sb[:, mc:mc + 1])
                        v1.append(vt)
                    # L2: 256 -> 128 (+ b2)
                    ps2 = ps512.tile([128, TILE], dt, tag="b512")
                    for kc in range(2):
                        nc.tensor.matmul(ps2[:], w2_sb[:, kc, :], v1[kc][:],
                                         start=(kc == 0), stop=(kc == 1))
                    pfe = wpool.tile([128, TILE], dt, tag="pfe")
                    nc.scalar.activation(pfe[:], ps2[:], AF.Identity,
                                         bias=b2_sb[:, 0:1])
                    # L3: 128 -> 64 (extended to 128 rows, rows 64.. = 0)
                    ps3 = ps512.tile([128, TILE], dt, tag="b512")
                    nc.tensor.matmul(ps3[:], s1_sb[:], pfe[:],
                                     start=True, stop=True)
                    h2 = wpool.tile([128, TILE], dt, tag="h2")
                    nc.scalar.activation(h2[:], ps3[:], AF.Relu,
                                         bias=sb1_sb[:, 0:1])
                    # per 128-cell block: transpose feats, logits column
                    stg = spool.tile([128, 4, GROW], dt, tag="stg")
                    for j in range(4):
                        blkcol = (b * CELLS_B + c0) // 128 + j
                        psT = ps128.tile([128, 128], dt, tag="b128")
                        nc.tensor.transpose(psT[:], pfe[:, j * 128:(j + 1) * 128],
                                            ident_sb[:])
                        nc.vector.tensor_copy(stg[:, j, 0:128], psT[:])
                        psL = ps128.tile([128, 128], dt, tag="b128")
                        nc.tensor.matmul(psL[:, 0:1],
                                         h2[:, j * 128:(j + 1) * 128],
                                         s2_sb[:], start=True, stop=True)
                        nc.scalar.activation(stg[:, j, 128:129], psL[:, 0:1],
                                             AF.Sigmoid, bias=sb2_sb[:, 0:1])
                        nc.scalar.activation(glog_sb[:, blkcol:blkcol + 1],
                                             psL[:, 0:1], AF.Identity,
                                             bias=sb2_sb[:, 0:1])
                        nc.vector.memset(stg[:, j, 129:GROW], 0.0)
                    row0 = b * CELLS_B + c0
                    dst = grid_T[row0:row0 + TILE, :].rearrange(
                        "(j p) c -> p j c", p=128)
                    nc.sync.dma_start(dst, stg[:])
            if "a" in PHASES:
                nc.sync.dma_start(gl_out[:, :], glog_sb[:])

            # ================= phase B: gather + segment reduce =================
            for p in range(NPAGES if "b" in PHASES else 0):
                gt = gpool.tile([128, nblk_page, GROW], dt, tag="gt")
                nc.gpsimd.dma_gather(
                    out_ap=gt[:],
                    in_ap=grid_T[:, :],
                    idxs_ap=gidx_sb[:, p * nblk_page * 8:(p + 1) * nblk_page * 8],
                    num_idxs=nblk_page * 128,
                    num_idxs_reg=nblk_page * 128,
                    elem_size=GROW,
                    single_packet=False,
                )
                # per-point prob into xyzp col 3
                nc.vector.tensor_copy(
                    xyzp_sb[:, p * nblk_page:(p + 1) * nblk_page, 3:4],
                    gt[:, :, 128:129])
                psF = ps128.tile([128, 128], dt, tag="b128")
                psX = ps128.tile([128, 128], dt, tag="b128")
                for bl in range(nblk_page):
                    blk = p * nblk_page + bl
                    st = wpool.tile([128, PAGE], dt, tag="stair")
                    nc.vector.tensor_scalar(
                        st[:], iota_sb[:, :], cval_sb[:, blk:blk + 1], None,
                        ALU.is_equal)
                    nc.tensor.matmul(psF[:], gt[:, bl, 0:128], st[:],
                                     start=(bl == 0), stop=(bl == nblk_page - 1))
                    nc.tensor.matmul(psX[:, 0:5], st[:], xyzp_sb[:, blk, :],
                                     start=(bl == 0), stop=(bl == nblk_page - 1))
                # counts -> reciprocal (clusters on partitions in psX)
                rec = wpool.tile([128, 1], dt, tag="rec")
                nc.vector.tensor_scalar_max(rec[:], psX[:, 4:5], 1.0)
                nc.vector.reciprocal(rec[:], rec[:])
                nc.vector.tensor_scalar_mul(c4st[:, p, :], psX[:, 0:4], rec[:])
                # feat means: transpose -> divide -> transpose back
                fsum = wpool.tile([128, 128], dt, tag="fsum")
                nc.vector.tensor_copy(fsum[:], psF[:])
                psT1 = ps128.tile([128, 128], dt, tag="b128")
                nc.tensor.transpose(psT1[:], fsum[:], ident_sb[:])
                mft = wpool.tile([128, 128], dt, tag="mft")
                nc.vector.tensor_scalar_mul(mft[:], psT1[:], rec[:])
                psT2 = ps128.tile([128, 128], dt, tag="b128")
                nc.tensor.transpose(psT2[:], mft[:], ident_sb[:])
                nc.vector.tensor_copy(meanF[:, p * 128:(p + 1) * 128], psT2[:])
                # point_feats out (sorted-slot order)
                r0 = p * nblk_page * 128
                dst = pf_out[r0:r0 + nblk_page * 128, :].rearrange(
                    "(bl q) c -> q bl c", q=128)
                nc.sync.dma_start(dst, gt[:, :, 0:128])
            if "b" in PHASES:
                nc.sync.dma_start(
                    c4_out[:, :].rearrange("(p q) c -> q p c", q=128), c4st[:])

            # ================= phase C: cluster MLP =================
            NSPLIT = [(0, 512), (512, 256)] if "c" in PHASES else []
            hC = ppool.tile([128, 2, CMAX], dt, tag="hC")
            for mc in range(2 if "c" in PHASES else 0):
                for (n0, nw) in NSPLIT:
                    ps = ps512.tile([128, TILE], dt, tag="b512")
                    nc.tensor.matmul(ps[:, 0:nw],
                                     p1_sb[:, mc * 128:(mc + 1) * 128],
                                     meanF[:, n0:n0 + nw], start=True, stop=True)
                    nc.scalar.activation(hC[:, mc, n0:n0 + nw], ps[:, 0:nw],
                                         AF.Relu, bias=pb1_sb[:, mc:mc + 1])
            pstg = ppool.tile([128, NPAGES, 768], dt, tag="pstg")
            for mc in range(6 if "c" in PHASES else 0):
                prj = wpool.tile([128, CMAX], dt, tag="prj")
                for (n0, nw) in NSPLIT:
                    ps = ps512.tile([128, TILE], dt, tag="b512")
                    for kc in range(2):
                        nc.tensor.matmul(ps[:, 0:nw],
                                         p2_sb[:, kc, mc * 128:(mc + 1) * 128],
                                         hC[:, kc, n0:n0 + nw],
                                         start=(kc == 0), stop=(kc == 1))
                    nc.scalar.activation(prj[:, n0:n0 + nw], ps[:, 0:nw],
                                         AF.Identity, bias=pb2_sb[:, mc:mc + 1])
                for cb in range(NPAGES):
                    psT = ps128.tile([128, 128], dt, tag="b128")
                    nc.tensor.transpose(psT[:], prj[:, cb * 128:(cb + 1) * 128],
                                        ident_sb[:])
                    nc.vector.tensor_copy(
                        pstg[:, cb, mc * 128:(mc + 1) * 128], psT[:])
            if "c" in PHASES:
                nc.sync.dma_start(
                    cp_out[:, :].rearrange("(cb q) c -> q cb c", q=128),
                    pstg[:])

    nc.compile()
    return nc


def _preprocess(inputs):
    """Host-side sharding/layout prep. Returns (in_maps, core_meta,
    nblk_page, rank_start, n_real)."""
    inp = {k: np.ascontiguousarray(np.asarray(v)) for k, v in inputs.items()}
    points = inp["points"].astype(f32, copy=False)
    batch_idx = inp["batch_idx"].astype(np.int32, copy=False)
    bev = inp["bev"].astype(f32, copy=False)

    # ---- fold BN into W1/b1 and S1/sb1 ----
    s1v = inp["g1"] / np.sqrt(inp["v1"] + BN_EPS)
    W1f = (inp["W1"] * s1v[None, :]).astype(f32)
    b1f = ((inp["b1"] - inp["m1"]) * s1v + inp["be1"]).astype(f32)
    s2v = inp["g2"] / np.sqrt(inp["v2"] + BN_EPS)
    S1f = (inp["S1"] * s2v[None, :]).astype(f32)
    sb1f = ((inp["sb1"] - inp["m2"]) * s2v + inp["be2"]).astype(f32)

    # ---- per-point indices (bit-exact mirror of the reference) ----
    gx = np.clip(np.floor((points[:, 0] - X0) / VOX).astype(np.int32), 0, W - 1)
    gy = np.clip(np.floor((points[:, 1] - X0) / VOX).astype(np.int32), 0, H - 1)
    cid = np.floor((points[:, :2] - X0) / CLUSTER).astype(np.int32)
    cinds = cid[:, 0] * 10000 + cid[:, 1] * 100 + batch_idx
    uniq, inv = np.unique(cinds, return_inverse=True)
    inv = inv.astype(np.int64)
    n_real = len(uniq)

    cid_of_rank = uniq // 10000
    core_of_rank = np.searchsorted(BOUNDS, cid_of_rank, side="right") - 1
    rank_start = [int(np.searchsorted(core_of_rank, i))
                  for i in range(NCORES + 1)]
    pt_order = np.argsort(inv, kind="stable")
    inv_sorted = inv[pt_order]

    # ---- per-core slot packing ----
    core_sel = []
    core_pages = []
    nblk_page = 1
    for i in range(NCORES):
        nclus = rank_start[i + 1] - rank_start[i]
        assert nclus <= CMAX, nclus
        lo = np.searchsorted(inv_sorted, rank_start[i], side="left")
        hi = np.searchsorted(inv_sorted, rank_start[i + 1], side="left")
        sel = pt_order[lo:hi]
        local_rank = inv[sel] - rank_start[i]
        page = local_rank // PAGE
        cnt = np.bincount(page, minlength=NPAGES)
        nblk_page = max(nblk_page, int(np.ceil(cnt.max() / 128)))
        core_sel.append((sel, local_rank, page))
        core_pages.append(cnt)
    nblk = NPAGES * nblk_page
    nslots = nblk * 128

    # ---- shared constant inputs ----
    eye = np.eye(128, dtype=f32)
    iota = np.broadcast_to(np.arange(PAGE, dtype=f32), (128, PAGE)).copy()
    w1_c = np.ascontiguousarray(W1f.reshape(3, 128, 256).transpose(1, 0, 2))
    b1_c = np.ascontiguousarray(b1f.reshape(2, 128).T)
    w2_c = np.ascontiguousarray(inp["W2"].reshape(2, 128, 128)
                                .transpose(1, 0, 2)).astype(f32)
    b2_c = inp["b2"].reshape(128, 1).astype(f32)
    s1_c = np.zeros((128, 128), f32)
    s1_c[:, 0:64] = S1f
    sb1_c = np.zeros((128, 1), f32)
    sb1_c[0:64, 0] = sb1f
    s2_c = np.zeros((128, 1), f32)
    s2_c[0:64, 0] = inp["S2"][:, 0]
    sb2_c = np.full((128, 1), inp["sb2"][0], f32)
    p1_c = inp["P1"].astype(f32)
    pb1_c = np.ascontiguousarray(inp["pb1"].reshape(2, 128).T).astype(f32)
    p2_c = np.ascontiguousarray(inp["P2"].reshape(2, 128, 768)
                                .transpose(1, 0, 2)).astype(f32)
    pb2_c = np.ascontiguousarray(inp["pb2"].reshape(6, 128).T).astype(f32)

    # ---- per-core inputs ----
    in_maps = []
    core_meta = []
    for i in range(NCORES):
        a, bnd = BOUNDS[i], BOUNDS[i + 1]
        x0 = max(0, 5 * a - 1)
        x1 = min(W, 5 * bnd + 1)
        assert x1 - x0 <= W_SLAB
        slab = np.zeros((B, C, H, W_SLAB), f32)
        slab[:, :, :, :x1 - x0] = bev[:, :, :, x0:x1]
        bev_c = np.ascontiguousarray(slab.reshape(B, 3, 128, H * W_SLAB))

        sel, local_rank, page = core_sel[i]
        slots = np.full(nslots, -1, np.int64)
        cval = np.full((nslots,), -1.0, f32)
        for p in range(NPAGES):
            pp = sel[page == p]
            base = p * nblk_page * 128
            slots[base:base + len(pp)] = pp
            cval[base:base + len(pp)] = (inv[pp] - rank_start[i]
                                         - p * PAGE).astype(f32)
        valid = slots >= 0
        vp = slots[valid]
        cell = np.zeros(nslots, np.int64)
        lx = gx[vp] - x0
        assert lx.min() >= 0 and lx.max() < W_SLAB, (lx.min(), lx.max())
        cell[valid] = (batch_idx[vp] * H + gy[vp]) * W_SLAB + lx

        # gather idx layout: index j of a page-call at [j%16, j//16],
        # 16-row pattern replicated to 128 partitions
        gidx = np.zeros((128, nblk * 8), np.int16)
        for p in range(NPAGES):
            pg = cell[p * nblk_page * 128:(p + 1) * nblk_page * 128]
            pat = pg.reshape(-1, 16).T.astype(np.int16)  # [16, nblk_page*8]
            gidx[:, p * nblk_page * 8:(p + 1) * nblk_page * 8] = np.tile(
                pat, (8, 1))

        # slot s = blk*128 + q  ->  [q, blk]
        cval_c = np.ascontiguousarray(
            cval.reshape(nblk, 128).T)
        xyzp = np.zeros((nslots, 5), f32)
        xyzp[valid, 0:3] = points[vp, 0:3]
        xyzp[valid, 4] = 1.0
        xyzp_c = np.ascontiguousarray(
            xyzp.reshape(nblk, 128, 5).transpose(1, 0, 2))

        in_maps.append(dict(
            bev_in=bev_c, gidx_in=gidx, cval_in=cval_c, xyzp_in=xyzp_c,
            iota_in=iota, ident_in=eye, w1_in=w1_c, b1_in=b1_c, w2_in=w2_c,
            b2_in=b2_c, s1_in=s1_c, sb1_in=sb1_c, s2_in=s2_c, sb2_in=sb2_c,
            p1_in=p1_c, pb1_in=pb1_c, p2_in=p2_c, pb2_in=pb2_c,
        ))
        core_meta.append((slots, valid, vp, cell))

    return in_maps, core_meta, nblk_page, rank_start, n_real


def _postprocess(inp, results, core_meta, rank_start, n_real):
    f32 = np.float32
    # ---- host assembly ----
    point_feats = np.zeros((N, 128), f32)
    seg_logits = np.zeros((N, 1), f32)
    cluster_proj = np.zeros((MAXC, 768), f32)
    cluster_xyz = np.zeros((MAXC, 3), f32)
    cluster_seg = np.zeros((MAXC, 1), f32)

    for i in range(NCORES):
        r = results[i]
        slots, valid, vp, cell = core_meta[i]
        point_feats[vp] = r["pf_out"][valid]
        gl = r["gl_out"]  # [128, CELLS//128]; cell c -> [c%128, c//128]
        cl = cell[valid]
        seg_logits[vp, 0] = gl[cl % 128, cl // 128]
        g0, g1 = rank_start[i], rank_start[i + 1]
        nr = g1 - g0
        cluster_proj[g0:g1] = r["cp_out"][:nr]
        cluster_xyz[g0:g1] = r["c4_out"][:nr, 0:3]
        cluster_seg[g0:g1, 0] = r["c4_out"][:nr, 3]

    # empty clusters: feats are exactly 0 -> proj row is a constant
    if n_real < MAXC:
        empty_row = (np.maximum(inp["pb1"].astype(np.float64), 0)
                     @ inp["P2"].astype(np.float64)
                     + inp["pb2"]).astype(f32)
        cluster_proj[n_real:] = empty_row[None, :]

    return point_feats, seg_logits, cluster_proj, cluster_xyz, cluster_seg


def kernel(**inputs):
    global LAST_RESULTS
    _install_ntff_shim()
    from concourse.bass_utils import run_bass_kernel_spmd

    inp = {k: np.ascontiguousarray(np.asarray(v)) for k, v in inputs.items()}
    in_maps, core_meta, nblk_page, rank_start, n_real = _preprocess(inp)
    nc = _build_program(nblk_page)
    res = run_bass_kernel_spmd(
        nc, in_maps, core_ids=list(range(NCORES)), trace=TRACE,
        trace_cores=list(range(NCORES)) if TRACE else None)
    LAST_RESULTS = res
    return _postprocess(inp, res.results, core_meta, rank_start, n_real)


# revision 21
# speedup vs baseline: 1.0920x; 1.0920x over previous
"""Trainium2 Bass kernel for PointPillarClusterFusion.

Key restructuring (validated against the reference in numpy):
  * The whole per-point MLP depends only on the point's voxel (b, gy, gx),
    so it is computed once per grid cell (grid MLP, BN folded into W1/S1),
    then per-point values are an irregular gather of grid rows.
  * Points are sorted by cluster id (= rank of cinds among sorted unique
    values); cluster segment-sums become small dense matmuls against 0/1
    "staircase" matrices built on-device with an is_equal DVE op.
  * Sharding: 8 contiguous cid_x ranges; each core gets a bev x-slab and
    all points/clusters of its range, so segment reductions stay local.

Per core (identical program, different data):
  A) grid MLP over its slab (384->256->128->64->1), write per-cell rows
     [feat128 | prob | pad] to a DRAM scratch (grid_T), logits to an output.
  B) per cluster-page (128 clusters): dma_gather of its points' grid rows,
     staircase matmuls accumulate feat/xyz/prob/count sums in PSUM, divide
     by counts via per-page PE transposes; gathered feat rows DMA out as
     the point_feats output (sorted order; host inverse-permutes).
  C) cluster MLP (128->256->768) on the cluster means, transpose, DMA out.
"""

import os
import sys
import numpy as np

# --- problem constants (from the reference) ---
VOX = np.float32(0.4)
X0 = np.float32(-51.2)
CLUSTER = np.float32(2.0)
MAXC = 8192
B, C, H, W = 2, 384, 256, 256
N = 200000
BN_EPS = np.float32(1e-5)

# --- sharding / layout constants ---
NCORES = 8
BOUNDS = [0, 7, 13, 20, 26, 33, 39, 46, 52]  # cid_x range per core
W_SLAB = 38
CELLS_B = H * W_SLAB            # 9728 cells per batch in the slab
CELLS = B * CELLS_B             # 19456
TILE = 512
NT_B = CELLS_B // TILE          # 19 tiles per batch
PAGE = 128                      # clusters per page
NPAGES = 6
CMAX = PAGE * NPAGES            # 768 cluster slots per core
GROW = 192                      # grid_T row floats (768 B, 256-multiple)

f32 = np.float32

TRACE = False           # set by test harness to capture an NTFF profile
LAST_RESULTS = None     # BassKernelResults of the last kernel() call


def _install_ntff_shim():
    """bass_utils imports antenv.axon_hooks when tracing; provide it via
    ctypes against libaxon_pjrt.so if the real module is absent."""
    try:
        import antenv.axon_hooks  # noqa: F401
        return
    except ImportError:
        pass
    import contextlib
    import ctypes
    import types

    so_path = "/opt/axon/libaxon_pjrt.so"
    hook = None
    try:
        lib = ctypes.CDLL(so_path)
        if hasattr(lib, "axon_start_nrt_profile"):
            lib.axon_start_nrt_profile.argtypes = [
                ctypes.POINTER(ctypes.c_int64),
                ctypes.c_size_t,
            ]
            lib.axon_start_nrt_profile.restype = ctypes.c_int64
            lib.axon_stop_nrt_profile.argtypes = [ctypes.c_char_p]
            lib.axon_stop_nrt_profile.restype = ctypes.c_int64

            @contextlib.contextmanager
            def _profile(output_dir, device_ids):
                import jax
                jax.devices()
                if device_ids:
                    ids = (ctypes.c_int64 * len(device_ids))(*device_ids)
                    rc = lib.axon_start_nrt_profile(ids, len(device_ids))
                else:
                    rc = lib.axon_start_nrt_profile(None, 0)
                if rc != 0:
                    raise RuntimeError(f"axon_start_nrt_profile rc={rc}")
                try:
                    yield
                finally:
                    n = lib.axon_stop_nrt_profile(str(output_dir).encode())
                    print(f"ntff profile: {n} file(s) -> {output_dir}",
                          file=sys.stderr)

            hook = _profile
    except OSError:
        pass

    mod = types.ModuleType("antenv.axon_hooks")
    mod.get_axon_ntff_profile_hook = lambda: hook
    mod.set_axon_ntff_profile_hook = lambda h: None
    try:
        import antenv
        antenv.axon_hooks = mod
    except ImportError:
        pass
    sys.modules["antenv.axon_hooks"] = mod


PHASES = "abc"  # bisection knob: which phases to emit


def _build_program(nblk_page):
    import concourse.bacc as bacc
    import concourse.mybir as mybir
    import concourse.tile as tile

    dt = mybir.dt.float32
    AF = mybir.ActivationFunctionType
    ALU = mybir.AluOpType
    nblk = NPAGES * nblk_page

    nc = bacc.Bacc("TRN2", target_bir_lowering=False, debug=False,
                   num_devices=NCORES, num_swdge_queues=4)

    # ---- external tensors ----
    bev_in = nc.dram_tensor("bev_in", [B, 3, 128, CELLS_B], dt,
                            kind="ExternalInput")
    gidx_in = nc.dram_tensor("gidx_in", [128, nblk * 8], mybir.dt.int16,
                             kind="ExternalInput")
    cval_in = nc.dram_tensor("cval_in", [128, nblk], dt, kind="ExternalInput")
    xyzp_in = nc.dram_tensor("xyzp_in", [128, nblk, 5], dt,
                             kind="ExternalInput")
    iota_in = nc.dram_tensor("iota_in", [128, PAGE], dt, kind="ExternalInput")
    ident_in = nc.dram_tensor("ident_in", [128, 128], dt,
                              kind="ExternalInput")
    w1_in = nc.dram_tensor("w1_in", [128, 3, 256], dt, kind="ExternalInput")
    b1_in = nc.dram_tensor("b1_in", [128, 2], dt, kind="ExternalInput")
    w2_in = nc.dram_tensor("w2_in", [128, 2, 128], dt, kind="ExternalInput")
    b2_in = nc.dram_tensor("b2_in", [128, 1], dt, kind="ExternalInput")
    s1_in = nc.dram_tensor("s1_in", [128, 128], dt, kind="ExternalInput")
    sb1_in = nc.dram_tensor("sb1_in", [128, 1], dt, kind="ExternalInput")
    s2_in = nc.dram_tensor("s2_in", [128, 1], dt, kind="ExternalInput")
    sb2_in = nc.dram_tensor("sb2_in", [128, 1], dt, kind="ExternalInput")
    p1_in = nc.dram_tensor("p1_in", [128, 256], dt, kind="ExternalInput")
    pb1_in = nc.dram_tensor("pb1_in", [128, 2], dt, kind="ExternalInput")
    p2_in = nc.dram_tensor("p2_in", [128, 2, 768], dt, kind="ExternalInput")
    pb2_in = nc.dram_tensor("pb2_in", [128, 6], dt, kind="ExternalInput")

    pf_out = nc.dram_tensor("pf_out", [nblk * 128, 128], dt,
                            kind="ExternalOutput")
    gl_out = nc.dram_tensor("gl_out", [128, CELLS // 128], dt,
                            kind="ExternalOutput")
    c4_out = nc.dram_tensor("c4_out", [CMAX, 4], dt, kind="ExternalOutput")
    cp_out = nc.dram_tensor("cp_out", [CMAX, 768], dt, kind="ExternalOutput")

    with tile.TileContext(nc) as tc:
        with (
            tc.tile_pool(name="const", bufs=1) as cpool,
            tc.tile_pool(name="work", bufs=2) as wpool,
            tc.tile_pool(name="stage", bufs=3) as spool,
            tc.tile_pool(name="gath", bufs=2) as gpool,
            tc.tile_pool(name="stairp", bufs=1) as stpool,
            tc.tile_pool(name="persist", bufs=1) as ppool,
            tc.tile_pool(name="ps512", bufs=4, space="PSUM") as ps512,
            tc.tile_pool(name="ps128", bufs=4, space="PSUM") as ps128,
            tc.tile_pool(name="dram", bufs=1, space="DRAM") as dpool,
        ):
            # ---- const preloads ----
            def load_const(name, ap, shape):
                t = cpool.tile(shape, ap.dtype, tag=name)
                nc.sync.dma_start(t[:], ap)
                return t

            w1_sb = load_const("w1", w1_in[:, :, :], [128, 3, 256])
            b1_sb = load_const("b1", b1_in[:, :], [128, 2])
            w2_sb = load_const("w2", w2_in[:, :, :], [128, 2, 128])
            b2_sb = load_const("b2", b2_in[:, :], [128, 1])
            s1_sb = load_const("s1", s1_in[:, :], [128, 128])
            sb1_sb = load_const("sb1", sb1_in[:, :], [128, 1])
            s2_sb = load_const("s2", s2_in[:, :], [128, 1])
            sb2_sb = load_const("sb2", sb2_in[:, :], [128, 1])
            p1_sb = load_const("p1", p1_in[:, :], [128, 256])
            pb1_sb = load_const("pb1", pb1_in[:, :], [128, 2])
            p2_sb = load_const("p2", p2_in[:, :, :], [128, 2, 768])
            pb2_sb = load_const("pb2", pb2_in[:, :], [128, 6])
            iota_sb = load_const("iota", iota_in[:, :], [128, PAGE])
            ident_sb = load_const("ident", ident_in[:, :], [128, 128])
            gidx_sb = load_const("gidx", gidx_in[:, :], [128, nblk * 8])
            cval_sb = load_const("cval", cval_in[:, :], [128, nblk])
            xyzp_sb = load_const("xyzp", xyzp_in[:, :, :], [128, nblk, 5])

            glog_sb = ppool.tile([128, CELLS // 128], dt, tag="glog")
            meanF = ppool.tile([128, CMAX], dt, tag="meanF")
            c4st = ppool.tile([128, NPAGES, 4], dt, tag="c4st")

            grid_T = dpool.tile([CELLS, GROW], dt, tag="gridT")

            # ================= phase A: grid MLP =================
            ctxA = nc.named_scope("phaseA")
            ctxA.__enter__()
            for b in range(B if "a" in PHASES else 0):
                for t in range(NT_B):
                    c0 = t * TILE
                    bevt = []
                    for ck in range(3):
                        bt = wpool.tile([128, TILE], dt, tag=f"bev{ck}")
                        nc.sync.dma_start(bt[:], bev_in[b, ck, :, c0:c0 + TILE])
                        bevt.append(bt)
                    # L1: 384 -> 256 (two M chunks), folded BN + relu
                    v1 = []
                    for mc in range(2):
                        ps = ps512.tile([128, TILE], dt, tag="b512")
                        for ck in range(3):
                            nc.tensor.matmul(
                                ps[:], w1_sb[:, ck, mc * 128:(mc + 1) * 128],
                                bevt[ck][:], start=(ck == 0), stop=(ck == 2))
                        vt = wpool.tile([128, TILE], dt, tag=f"v1{mc}")
                        nc.scalar.activation(vt[:], ps[:], AF.Relu,
                                             bias=b1_sb[:, mc:mc + 1])
                        v1.append(vt)
                    # L2: 256 -> 128 (+ b2)
                    ps2 = ps512.tile([128, TILE], dt, tag="b512")
                    for kc in range(2):
                        nc.tensor.matmul(ps2[:], w2_sb[:, kc, :], v1[kc][:],
                                         start=(kc == 0), stop=(kc == 1))
                    pfe = wpool.tile([128, TILE], dt, tag="pfe")
                    nc.scalar.activation(pfe[:], ps2[:], AF.Identity,
                                         bias=b2_sb[:, 0:1])
                    # L3: 128 -> 64 (extended to 128 rows, rows 64.. = 0)
                    ps3 = ps512.tile([128, TILE], dt, tag="b512")
                    nc.tensor.matmul(ps3[:], s1_sb[:], pfe[:],
                                     start=True, stop=True)
                    h2 = wpool.tile([128, TILE], dt, tag="h2")
                    nc.scalar.activation(h2[:], ps3[:], AF.Relu,
                                         bias=sb1_sb[:, 0:1])
                    # per 128-cell block: transpose feats, logits column
                    stg = spool.tile([128, 4, GROW], dt, tag="stg")
                    for j in range(4):
                        blkcol = (b * CELLS_B + c0) // 128 + j
                        psT = ps128.tile([128, 128], dt, tag="b128")
                        nc.tensor.transpose(psT[:], pfe[:, j * 128:(j + 1) * 128],
                                            ident_sb[:])
                        nc.vector.tensor_copy(stg[:, j, 0:128], psT[:])
                        psL = ps128.tile([128, 128], dt, tag="b128")
                        nc.tensor.matmul(psL[:, 0:1],
                                         h2[:, j * 128:(j + 1) * 128],
                                         s2_sb[:], start=True, stop=True)
                        nc.scalar.activation(stg[:, j, 128:129], psL[:, 0:1],
                                             AF.Sigmoid, bias=sb2_sb[:, 0:1])
                        nc.scalar.activation(glog_sb[:, blkcol:blkcol + 1],
                                             psL[:, 0:1], AF.Identity,
                                             bias=sb2_sb[:, 0:1])
                        nc.vector.memset(stg[:, j, 129:GROW], 0.0)
                    row0 = b * CELLS_B + c0
                    dst = grid_T[row0:row0 + TILE, :].rearrange(
                        "(j p) c -> p j c", p=128)
                    nc.sync.dma_start(dst, stg[:])
            if "a" in PHASES:
                nc.sync.dma_start(gl_out[:, :], glog_sb[:])
            ctxA.__exit__(None, None, None)

            # ================= phase B: gather + segment reduce =================
            ctxB = nc.named_scope("phaseB")
            ctxB.__enter__()
            for p in range(NPAGES if "b" in PHASES else 0):
                gt = gpool.tile([128, nblk_page, GROW], dt, tag="gt")
                nc.gpsimd.dma_gather(
                    out_ap=gt[:],
                    in_ap=grid_T[:, :],
                    idxs_ap=gidx_sb[:, p * nblk_page * 8:(p + 1) * nblk_page * 8],
                    num_idxs=nblk_page * 128,
                    num_idxs_reg=nblk_page * 128,
                    elem_size=GROW,
                    single_packet=False,
                    queue_num=p % 4,
                )
                # per-point prob into xyzp col 3
                nc.vector.tensor_copy(
                    xyzp_sb[:, p * nblk_page:(p + 1) * nblk_page, 3:4],
                    gt[:, :, 128:129])
                # staircase for the whole page in one DVE op:
                # stair[q, bl, j] = (cval[q, p*nbp+bl] == iota[j])
                stair = stpool.tile([128, nblk_page, PAGE], dt, tag="stair")
                nc.vector.scalar_tensor_tensor(
                    stair[:],
                    cval_sb[:, p * nblk_page:(p + 1) * nblk_page, None]
                    .to_broadcast((128, nblk_page, PAGE)),
                    0.0,
                    iota_sb[:, None, :].to_broadcast((128, nblk_page, PAGE)),
                    ALU.bypass, ALU.is_equal)
                psF = ps128.tile([128, 128], dt, tag="b128")
                psX = ps128.tile([128, 128], dt, tag="b128")
                for bl in range(nblk_page):
                    blk = p * nblk_page + bl
                    st = stair[:, bl, :]
                    nc.tensor.matmul(psF[:], gt[:, bl, 0:128], st,
                                     start=(bl == 0), stop=(bl == nblk_page - 1))
                    nc.tensor.matmul(psX[:, 0:5], st, xyzp_sb[:, blk, :],
                                     start=(bl == 0), stop=(bl == nblk_page - 1))
                # counts -> reciprocal (clusters on partitions in psX)
                rec = wpool.tile([128, 1], dt, tag="rec")
                nc.vector.tensor_scalar_max(rec[:], psX[:, 4:5], 1.0)
                nc.vector.reciprocal(rec[:], rec[:])
                nc.vector.tensor_scalar_mul(c4st[:, p, :], psX[:, 0:4], rec[:])
                # feat means: transpose -> divide -> transpose back
                fsum = wpool.tile([128, 128], dt, tag="fsum")
                nc.vector.tensor_copy(fsum[:], psF[:])
                psT1 = ps128.tile([128, 128], dt, tag="b128")
                nc.tensor.transpose(psT1[:], fsum[:], ident_sb[:])
                mft = wpool.tile([128, 128], dt, tag="mft")
                nc.vector.tensor_scalar_mul(mft[:], psT1[:], rec[:])
                psT2 = ps128.tile([128, 128], dt, tag="b128")
                nc.tensor.transpose(psT2[:], mft[:], ident_sb[:])
                nc.vector.tensor_copy(meanF[:, p * 128:(p + 1) * 128], psT2[:])
                # point_feats out (sorted-slot order)
                r0 = p * nblk_page * 128
                dst = pf_out[r0:r0 + nblk_page * 128, :].rearrange(
                    "(bl q) c -> q bl c", q=128)
                nc.sync.dma_start(dst, gt[:, :, 0:128])
            if "b" in PHASES:
                nc.sync.dma_start(
                    c4_out[:, :].rearrange("(p q) c -> q p c", q=128), c4st[:])
            ctxB.__exit__(None, None, None)

            # ================= phase C: cluster MLP =================
            ctxC = nc.named_scope("phaseC")
            ctxC.__enter__()
            NSPLIT = [(0, 512), (512, 256)] if "c" in PHASES else []
            hC = ppool.tile([128, 2, CMAX], dt, tag="hC")
            for mc in range(2 if "c" in PHASES else 0):
                for (n0, nw) in NSPLIT:
                    ps = ps512.tile([128, TILE], dt, tag="b512")
                    nc.tensor.matmul(ps[:, 0:nw],
                                     p1_sb[:, mc * 128:(mc + 1) * 128],
                                     meanF[:, n0:n0 + nw], start=True, stop=True)
                    nc.scalar.activation(hC[:, mc, n0:n0 + nw], ps[:, 0:nw],
                                         AF.Relu, bias=pb1_sb[:, mc:mc + 1])
            pstg = ppool.tile([128, NPAGES, 768], dt, tag="pstg")
            for mc in range(6 if "c" in PHASES else 0):
                prj = wpool.tile([128, CMAX], dt, tag="prj")
                for (n0, nw) in NSPLIT:
                    ps = ps512.tile([128, TILE], dt, tag="b512")
                    for kc in range(2):
                        nc.tensor.matmul(ps[:, 0:nw],
                                         p2_sb[:, kc, mc * 128:(mc + 1) * 128],
                                         hC[:, kc, n0:n0 + nw],
                                         start=(kc == 0), stop=(kc == 1))
                    nc.scalar.activation(prj[:, n0:n0 + nw], ps[:, 0:nw],
                                         AF.Identity, bias=pb2_sb[:, mc:mc + 1])
                for cb in range(NPAGES):
                    psT = ps128.tile([128, 128], dt, tag="b128")
                    nc.tensor.transpose(psT[:], prj[:, cb * 128:(cb + 1) * 128],
                                        ident_sb[:])
                    nc.vector.tensor_copy(
                        pstg[:, cb, mc * 128:(mc + 1) * 128], psT[:])
            if "c" in PHASES:
                nc.sync.dma_start(
                    cp_out[:, :].rearrange("(cb q) c -> q cb c", q=128),
                    pstg[:])
            ctxC.__exit__(None, None, None)

    nc.compile()
    return nc


def _preprocess(inputs):
    """Host-side sharding/layout prep. Returns (in_maps, core_meta,
    nblk_page, rank_start, n_real)."""
    inp = {k: np.ascontiguousarray(np.asarray(v)) for k, v in inputs.items()}
    points = inp["points"].astype(f32, copy=False)
    batch_idx = inp["batch_idx"].astype(np.int32, copy=False)
    bev = inp["bev"].astype(f32, copy=False)

    # ---- fold BN into W1/b1 and S1/sb1 ----
    s1v = inp["g1"] / np.sqrt(inp["v1"] + BN_EPS)
    W1f = (inp["W1"] * s1v[None, :]).astype(f32)
    b1f = ((inp["b1"] - inp["m1"]) * s1v + inp["be1"]).astype(f32)
    s2v = inp["g2"] / np.sqrt(inp["v2"] + BN_EPS)
    S1f = (inp["S1"] * s2v[None, :]).astype(f32)
    sb1f = ((inp["sb1"] - inp["m2"]) * s2v + inp["be2"]).astype(f32)

    # ---- per-point indices (bit-exact mirror of the reference) ----
    gx = np.clip(np.floor((points[:, 0] - X0) / VOX).astype(np.int32), 0, W - 1)
    gy = np.clip(np.floor((points[:, 1] - X0) / VOX).astype(np.int32), 0, H - 1)
    cid = np.floor((points[:, :2] - X0) / CLUSTER).astype(np.int32)
    cinds = cid[:, 0] * 10000 + cid[:, 1] * 100 + batch_idx
    uniq, inv = np.unique(cinds, return_inverse=True)
    inv = inv.astype(np.int64)
    n_real = len(uniq)

    cid_of_rank = uniq // 10000
    core_of_rank = np.searchsorted(BOUNDS, cid_of_rank, side="right") - 1
    rank_start = [int(np.searchsorted(core_of_rank, i))
                  for i in range(NCORES + 1)]
    pt_order = np.argsort(inv, kind="stable")
    inv_sorted = inv[pt_order]

    # ---- per-core slot packing ----
    core_sel = []
    core_pages = []
    nblk_page = 1
    for i in range(NCORES):
        nclus = rank_start[i + 1] - rank_start[i]
        assert nclus <= CMAX, nclus
        lo = np.searchsorted(inv_sorted, rank_start[i], side="left")
        hi = np.searchsorted(inv_sorted, rank_start[i + 1], side="left")
        sel = pt_order[lo:hi]
        local_rank = inv[sel] - rank_start[i]
        page = local_rank // PAGE
        cnt = np.bincount(page, minlength=NPAGES)
        nblk_page = max(nblk_page, int(np.ceil(cnt.max() / 128)))
        core_sel.append((sel, local_rank, page))
        core_pages.append(cnt)
    nblk = NPAGES * nblk_page
    nslots = nblk * 128

    # ---- shared constant inputs ----
    eye = np.eye(128, dtype=f32)
    iota = np.broadcast_to(np.arange(PAGE, dtype=f32), (128, PAGE)).copy()
    w1_c = np.ascontiguousarray(W1f.reshape(3, 128, 256).transpose(1, 0, 2))
    b1_c = np.ascontiguousarray(b1f.reshape(2, 128).T)
    w2_c = np.ascontiguousarray(inp["W2"].reshape(2, 128, 128)
                                .transpose(1, 0, 2)).astype(f32)
    b2_c = inp["b2"].reshape(128, 1).astype(f32)
    s1_c = np.zeros((128, 128), f32)
    s1_c[:, 0:64] = S1f
    sb1_c = np.zeros((128, 1), f32)
    sb1_c[0:64, 0] = sb1f
    s2_c = np.zeros((128, 1), f32)
    s2_c[0:64, 0] = inp["S2"][:, 0]
    sb2_c = np.full((128, 1), inp["sb2"][0], f32)
    p1_c = inp["P1"].astype(f32)
    pb1_c = np.ascontiguousarray(inp["pb1"].reshape(2, 128).T).astype(f32)
    p2_c = np.ascontiguousarray(inp["P2"].reshape(2, 128, 768)
                                .transpose(1, 0, 2)).astype(f32)
    pb2_c = np.ascontiguousarray(inp["pb2"].reshape(6, 128).T).astype(f32)

    # ---- per-core inputs ----
    in_maps = []
    core_meta = []
    for i in range(NCORES):
        a, bnd = BOUNDS[i], BOUNDS[i + 1]
        x0 = max(0, 5 * a - 1)
        x1 = min(W, 5 * bnd + 1)
        assert x1 - x0 <= W_SLAB
        slab = np.zeros((B, C, H, W_SLAB), f32)
        slab[:, :, :, :x1 - x0] = bev[:, :, :, x0:x1]
        bev_c = np.ascontiguousarray(slab.reshape(B, 3, 128, H * W_SLAB))

        sel, local_rank, page = core_sel[i]
        slots = np.full(nslots, -1, np.int64)
        cval = np.full((nslots,), -1.0, f32)
        for p in range(NPAGES):
            pp = sel[page == p]
            base = p * nblk_page * 128
            slots[base:base + len(pp)] = pp
            cval[base:base + len(pp)] = (inv[pp] - rank_start[i]
                                         - p * PAGE).astype(f32)
        valid = slots >= 0
        vp = slots[valid]
        cell = np.zeros(nslots, np.int64)
        lx = gx[vp] - x0
        assert lx.min() >= 0 and lx.max() < W_SLAB, (lx.min(), lx.max())
        cell[valid] = (batch_idx[vp] * H + gy[vp]) * W_SLAB + lx

        # gather idx layout: index j of a page-call at [j%16, j//16],
        # 16-row pattern replicated to 128 partitions
        gidx = np.zeros((128, nblk * 8), np.int16)
        for p in range(NPAGES):
            pg = cell[p * nblk_page * 128:(p + 1) * nblk_page * 128]
            pat = pg.reshape(-1, 16).T.astype(np.int16)  # [16, nblk_page*8]
            gidx[:, p * nblk_page * 8:(p + 1) * nblk_page * 8] = np.tile(
                pat, (8, 1))

        # slot s = blk*128 + q  ->  [q, blk]
        cval_c = np.ascontiguousarray(
            cval.reshape(nblk, 128).T)
        xyzp = np.zeros((nslots, 5), f32)
        xyzp[valid, 0:3] = points[vp, 0:3]
        xyzp[valid, 4] = 1.0
        xyzp_c = np.ascontiguousarray(
            xyzp.reshape(nblk, 128, 5).transpose(1, 0, 2))

        in_maps.append(dict(
            bev_in=bev_c, gidx_in=gidx, cval_in=cval_c, xyzp_in=xyzp_c,
            iota_in=iota, ident_in=eye, w1_in=w1_c, b1_in=b1_c, w2_in=w2_c,
            b2_in=b2_c, s1_in=s1_c, sb1_in=sb1_c, s2_in=s2_c, sb2_in=sb2_c,
            p1_in=p1_c, pb1_in=pb1_c, p2_in=p2_c, pb2_in=pb2_c,
        ))
        core_meta.append((slots, valid, vp, cell))

    return in_maps, core_meta, nblk_page, rank_start, n_real


def _postprocess(inp, results, core_meta, rank_start, n_real):
    f32 = np.float32
    # ---- host assembly ----
    point_feats = np.zeros((N, 128), f32)
    seg_logits = np.zeros((N, 1), f32)
    cluster_proj = np.zeros((MAXC, 768), f32)
    cluster_xyz = np.zeros((MAXC, 3), f32)
    cluster_seg = np.zeros((MAXC, 1), f32)

    for i in range(NCORES):
        r = results[i]
        slots, valid, vp, cell = core_meta[i]
        point_feats[vp] = r["pf_out"][valid]
        gl = r["gl_out"]  # [128, CELLS//128]; cell c -> [c%128, c//128]
        cl = cell[valid]
        seg_logits[vp, 0] = gl[cl % 128, cl // 128]
        g0, g1 = rank_start[i], rank_start[i + 1]
        nr = g1 - g0
        cluster_proj[g0:g1] = r["cp_out"][:nr]
        cluster_xyz[g0:g1] = r["c4_out"][:nr, 0:3]
        cluster_seg[g0:g1, 0] = r["c4_out"][:nr, 3]

    # empty clusters: feats are exactly 0 -> proj row is a constant
    if n_real < MAXC:
        empty_row = (np.maximum(inp["pb1"].astype(np.float64), 0)
                     @ inp["P2"].astype(np.float64)
                     + inp["pb2"]).astype(f32)
        cluster_proj[n_real:] = empty_row[None, :]

    return point_feats, seg_logits, cluster_proj, cluster_xyz, cluster_seg


def kernel(**inputs):
    global LAST_RESULTS
    _install_ntff_shim()
    from concourse.bass_utils import run_bass_kernel_spmd

    inp = {k: np.ascontiguousarray(np.asarray(v)) for k, v in inputs.items()}
    in_maps, core_meta, nblk_page, rank_start, n_real = _preprocess(inp)
    nc = _build_program(nblk_page)
    res = run_bass_kernel_spmd(
        nc, in_maps, core_ids=list(range(NCORES)), trace=TRACE,
        trace_cores=list(range(NCORES)) if TRACE else None)
    LAST_RESULTS = res
    return _postprocess(inp, res.results, core_meta, rank_start, n_real)


# revision 28
# speedup vs baseline: 1.6163x; 1.4801x over previous
"""Trainium2 Bass kernel for PointPillarClusterFusion.

Restructuring (validated vs the reference in numpy + CoreSim):
  * The per-point MLP depends only on the point's voxel (b, gy, gx): compute
    it once per grid cell (BN folded), then per-point values are a dma_gather
    of grid rows. Grid rows: [feat128 | prob | 1.0 | pad] bf16 (512 B).
  * Points sorted by cluster id (rank of cinds); segment sums are dense
    staircase matmuls (0/1 matrices built on-device with is_equal).
  * Sharding: 8 contiguous cid_x ranges -> per-core bev x-slab; cluster
    reductions stay core-local. Cells ordered x-major so each cluster page
    maps to a contiguous band of grid rows; the per-page gather reads only
    its band, which lets the Tile scheduler overlap phase B with phase A.
  * PE compute in bf16 (fp32 matmul streams ~4x slower on TRN2), PSUM fp32.
"""

import sys
import numpy as np

# --- problem constants (from the reference) ---
VOX = np.float32(0.4)
X0 = np.float32(-51.2)
CLUSTER = np.float32(2.0)
MAXC = 8192
B, C, H, W = 2, 384, 256, 256
N = 200000
BN_EPS = np.float32(1e-5)

# --- sharding / layout constants ---
NCORES = 8
BOUNDS = [0, 7, 13, 20, 26, 33, 39, 46, 52]  # cid_x range per core
W_SLAB = 38
CELLS = W_SLAB * B * H          # 19456, x-major: cell = (xl*2+b)*256+gy
TILE = 512                      # one x-column (b, gy) per tile
NT = W_SLAB                     # 38 tiles
PAGE = 128                      # clusters per page
NPAGES = 6
CMAX = PAGE * NPAGES            # 768 cluster slots per core
GROW = 256                      # grid_T row elements (bf16 -> 512 B)

f32 = np.float32

TRACE = False
PHASES = "abc"
LAST_RESULTS = None


def _install_ntff_shim():
    try:
        import antenv.axon_hooks  # noqa: F401
        return
    except ImportError:
        pass
    import contextlib
    import ctypes
    import types

    hook = None
    try:
        lib = ctypes.CDLL("/opt/axon/libaxon_pjrt.so")
        if hasattr(lib, "axon_start_nrt_profile"):
            lib.axon_start_nrt_profile.argtypes = [
                ctypes.POINTER(ctypes.c_int64), ctypes.c_size_t]
            lib.axon_start_nrt_profile.restype = ctypes.c_int64
            lib.axon_stop_nrt_profile.argtypes = [ctypes.c_char_p]
            lib.axon_stop_nrt_profile.restype = ctypes.c_int64

            @contextlib.contextmanager
            def _profile(output_dir, device_ids):
                import jax
                jax.devices()
                if device_ids:
                    ids = (ctypes.c_int64 * len(device_ids))(*device_ids)
                    rc = lib.axon_start_nrt_profile(ids, len(device_ids))
                else:
                    rc = lib.axon_start_nrt_profile(None, 0)
                if rc != 0:
                    raise RuntimeError(f"axon_start_nrt_profile rc={rc}")
                try:
                    yield
                finally:
                    n = lib.axon_stop_nrt_profile(str(output_dir).encode())
                    print(f"ntff profile: {n} file(s) -> {output_dir}",
                          file=sys.stderr)

            hook = _profile
    except OSError:
        pass

    mod = types.ModuleType("antenv.axon_hooks")
    mod.get_axon_ntff_profile_hook = lambda: hook
    mod.set_axon_ntff_profile_hook = lambda h: None
    try:
        import antenv
        antenv.axon_hooks = mod
    except ImportError:
        pass
    sys.modules["antenv.axon_hooks"] = mod


def _build_program(nblk_page, bands):
    """bands: per page (row_lo, row_hi) into grid_T, uniform across cores."""
    import concourse.bacc as bacc
    import concourse.mybir as mybir
    import concourse.tile as tile

    bf = mybir.dt.bfloat16
    fp = mybir.dt.float32
    AF = mybir.ActivationFunctionType
    ALU = mybir.AluOpType
    nblk = NPAGES * nblk_page

    nc = bacc.Bacc("TRN2", target_bir_lowering=False, debug=False,
                   num_devices=NCORES)

    bev_in = nc.dram_tensor("bev_in", [NT, 3, 128, TILE], bf,
                            kind="ExternalInput")
    gidx_in = nc.dram_tensor("gidx_in", [128, nblk * 8], mybir.dt.int16,
                             kind="ExternalInput")
    cval_in = nc.dram_tensor("cval_in", [128, nblk], bf, kind="ExternalInput")
    xyzp_in = nc.dram_tensor("xyzp_in", [128, nblk, 3], bf,
                             kind="ExternalInput")
    iota_in = nc.dram_tensor("iota_in", [128, PAGE], bf, kind="ExternalInput")
    zer_in = nc.dram_tensor("zer_in", [128, 8], bf, kind="ExternalInput")
    idbf_in = nc.dram_tensor("idbf_in", [128, 128], bf, kind="ExternalInput")
    idfp_in = nc.dram_tensor("idfp_in", [128, 128], fp, kind="ExternalInput")
    w1_in = nc.dram_tensor("w1_in", [128, 3, 256], bf, kind="ExternalInput")
    b1_in = nc.dram_tensor("b1_in", [128, 2], fp, kind="ExternalInput")
    w2_in = nc.dram_tensor("w2_in", [128, 2, 128], bf, kind="ExternalInput")
    b2_in = nc.dram_tensor("b2_in", [128, 1], fp, kind="ExternalInput")
    s1_in = nc.dram_tensor("s1_in", [128, 128], bf, kind="ExternalInput")
    sb1_in = nc.dram_tensor("sb1_in", [128, 1], fp, kind="ExternalInput")
    s2_in = nc.dram_tensor("s2_in", [128, 1], bf, kind="ExternalInput")
    sb2_in = nc.dram_tensor("sb2_in", [128, 1], fp, kind="ExternalInput")
    p1_in = nc.dram_tensor("p1_in", [128, 256], bf, kind="ExternalInput")
    pb1_in = nc.dram_tensor("pb1_in", [128, 2], fp, kind="ExternalInput")
    p2_in = nc.dram_tensor("p2_in", [128, 2, 768], bf, kind="ExternalInput")
    pb2_in = nc.dram_tensor("pb2_in", [128, 6], fp, kind="ExternalInput")

    pf_out = nc.dram_tensor("pf_out", [nblk * 128, 128], bf,
                            kind="ExternalOutput")
    gl_out = nc.dram_tensor("gl_out", [128, CELLS // 128], fp,
                            kind="ExternalOutput")
    c4_out = nc.dram_tensor("c4_out", [CMAX, 4], fp, kind="ExternalOutput")
    cp_out = nc.dram_tensor("cp_out", [CMAX, 768], fp, kind="ExternalOutput")

    with tile.TileContext(nc) as tc:
        with (
            tc.tile_pool(name="const", bufs=1) as cpool,
            tc.tile_pool(name="work", bufs=3) as wpool,
            tc.tile_pool(name="stage", bufs=3) as spool,
            tc.tile_pool(name="gath", bufs=2) as gpool,
            tc.tile_pool(name="stairp", bufs=2) as stpool,
            tc.tile_pool(name="persist", bufs=1) as ppool,
            tc.tile_pool(name="ps512", bufs=4, space="PSUM") as ps512,
            tc.tile_pool(name="ps128", bufs=4, space="PSUM") as ps128,
            tc.tile_pool(name="dram", bufs=1, space="DRAM") as dpool,
        ):
            def load_const(name, ap, shape, dtype):
                t = cpool.tile(shape, dtype, tag=name, name=name)
                nc.sync.dma_start(t[:], ap)
                return t

            w1_sb = load_const("w1", w1_in[:, :, :], [128, 3, 256], bf)
            b1_sb = load_const("b1", b1_in[:, :], [128, 2], fp)
            w2_sb = load_const("w2", w2_in[:, :, :], [128, 2, 128], bf)
            b2_sb = load_const("b2", b2_in[:, :], [128, 1], fp)
            s1_sb = load_const("s1", s1_in[:, :], [128, 128], bf)
            sb1_sb = load_const("sb1", sb1_in[:, :], [128, 1], fp)
            s2_sb = load_const("s2", s2_in[:, :], [128, 1], bf)
            sb2_sb = load_const("sb2", sb2_in[:, :], [128, 1], fp)
            p1_sb = load_const("p1", p1_in[:, :], [128, 256], bf)
            pb1_sb = load_const("pb1", pb1_in[:, :], [128, 2], fp)
            p2_sb = load_const("p2", p2_in[:, :, :], [128, 2, 768], bf)
            pb2_sb = load_const("pb2", pb2_in[:, :], [128, 6], fp)
            iota_sb = load_const("iota", iota_in[:, :], [128, PAGE], bf)
            zer_sb = load_const("zer", zer_in[:, :], [128, 8], bf)
            idbf_sb = load_const("idbf", idbf_in[:, :], [128, 128], bf)
            idfp_sb = load_const("idfp", idfp_in[:, :], [128, 128], fp)
            gidx_sb = load_const("gidx", gidx_in[:, :], [128, nblk * 8],
                                 mybir.dt.int16)
            cval_sb = load_const("cval", cval_in[:, :], [128, nblk], bf)
            xyzp_sb = load_const("xyzp", xyzp_in[:, :, :], [128, nblk, 3], bf)

            glog_sb = ppool.tile([128, CELLS // 128], fp, tag="glog",
                                 name="glog")
            meanF = ppool.tile([128, CMAX], bf, tag="meanF", name="meanF")
            c4st = ppool.tile([128, NPAGES, 4], fp, tag="c4st", name="c4st")

            grid_T = dpool.tile([CELLS, GROW], bf, tag="gridT", name="gridT")

            # ================= phase A: grid MLP (x-major tiles) ==========
            for t in range(NT if "a" in PHASES else 0):
                bevt = []
                for ck in range(3):
                    bt = wpool.tile([128, TILE], bf, tag=f"bev{ck}",
                                    name=f"bev{ck}_{t}")
                    nc.sync.dma_start(bt[:], bev_in[t, ck, :, :])
                    bevt.append(bt)
                v1 = []
                for mc in range(2):
                    ps = ps512.tile([128, TILE], fp, tag="b512",
                                    name=f"ps1{mc}_{t}")
                    for ck in range(3):
                        nc.tensor.matmul(
                            ps[:], w1_sb[:, ck, mc * 128:(mc + 1) * 128],
                            bevt[ck][:], start=(ck == 0), stop=(ck == 2))
                    vt = wpool.tile([128, TILE], bf, tag=f"v1{mc}",
                                    name=f"v1{mc}_{t}")
                    nc.scalar.activation(vt[:], ps[:], AF.Relu,
                                         bias=b1_sb[:, mc:mc + 1])
                    v1.append(vt)
                ps2 = ps512.tile([128, TILE], fp, tag="b512", name=f"ps2_{t}")
                for kc in range(2):
                    nc.tensor.matmul(ps2[:], w2_sb[:, kc, :], v1[kc][:],
                                     start=(kc == 0), stop=(kc == 1))
                pfe = wpool.tile([128, TILE], bf, tag="pfe", name=f"pfe_{t}")
                nc.scalar.activation(pfe[:], ps2[:], AF.Identity,
                                     bias=b2_sb[:, 0:1])
                ps3 = ps512.tile([128, TILE], fp, tag="b512", name=f"ps3_{t}")
                nc.tensor.matmul(ps3[:], s1_sb[:], pfe[:], start=True,
                                 stop=True)
                h2 = wpool.tile([128, TILE], bf, tag="h2", name=f"h2_{t}")
                nc.scalar.activation(h2[:], ps3[:], AF.Relu,
                                     bias=sb1_sb[:, 0:1])
                stg = spool.tile([128, 4, GROW], bf, tag="stg", name=f"stg{t}")
                for j in range(4):
                    blkcol = t * 4 + j
                    psT = ps128.tile([128, 128], bf, tag="b128",
                                     name=f"psT_{t}_{j}")
                    nc.tensor.transpose(psT[:], pfe[:, j * 128:(j + 1) * 128],
                                        idbf_sb[:])
                    nc.vector.tensor_copy(stg[:, j, 0:128], psT[:])
                    psL = ps128.tile([128, 128], fp, tag="b128",
                                     name=f"psL_{t}_{j}")
                    nc.tensor.matmul(psL[:, 0:1],
                                     h2[:, j * 128:(j + 1) * 128],
                                     s2_sb[:], start=True, stop=True)
                    nc.scalar.activation(stg[:, j, 128:129], psL[:, 0:1],
                                         AF.Sigmoid, bias=sb2_sb[:, 0:1])
                    nc.scalar.activation(glog_sb[:, blkcol:blkcol + 1],
                                         psL[:, 0:1], AF.Identity,
                                         bias=sb2_sb[:, 0:1])
                    nc.vector.memset(stg[:, j, 129:130], 1.0)
                    nc.vector.memset(stg[:, j, 130:GROW], 0.0)
                dst = grid_T[t * TILE:(t + 1) * TILE, :].rearrange(
                    "(j p) c -> p j c", p=128)
                nc.sync.dma_start(dst, stg[:])
            if "a" in PHASES:
                nc.sync.dma_start(gl_out[:, :], glog_sb[:])

            # ================= phase B: gather + segment reduce ==========
            for p in range(NPAGES if "b" in PHASES else 0):
                lo, hi = bands[p]
                gt = gpool.tile([128, nblk_page, GROW], bf, tag="gt",
                                name=f"gt{p}")
                nc.gpsimd.dma_gather(
                    out_ap=gt[:],
                    in_ap=grid_T[lo:hi, :],
                    idxs_ap=gidx_sb[:, p * nblk_page * 8:(p + 1) * nblk_page * 8],
                    num_idxs=nblk_page * 128,
                    num_idxs_reg=nblk_page * 128,
                    elem_size=GROW,
                    single_packet=False,
                )
                stair = stpool.tile([128, nblk_page, PAGE], bf, tag="stair",
                                    name=f"stair{p}")
                nc.vector.scalar_tensor_tensor(
                    stair[:],
                    cval_sb[:, p * nblk_page:(p + 1) * nblk_page, None]
                    .to_broadcast((128, nblk_page, PAGE)),
                    0.0,
                    iota_sb[:, None, :].to_broadcast((128, nblk_page, PAGE)),
                    ALU.bypass, ALU.is_equal)
                psF = ps128.tile([128, 128], fp, tag="b128", name=f"psF{p}")
                psX = ps128.tile([128, 128], fp, tag="b128", name=f"psX{p}")
                # psum zeroing (start flag) is zero-region-granular, so the
                # xyz and prob/cnt accumulations in one bank must share one
                # group: open/close it with zero-matmuls over the full range
                nc.tensor.matmul(psX[:, 0:5], iota_sb[:, :], zer_sb[:, 0:5],
                                 start=True, stop=False)
                for bl in range(nblk_page):
                    blk = p * nblk_page + bl
                    st = stair[:, bl, :]
                    last = bl == nblk_page - 1
                    nc.tensor.matmul(psF[:], gt[:, bl, 0:128], st,
                                     start=(bl == 0), stop=last)
                    nc.tensor.matmul(psX[:, 0:3], st, xyzp_sb[:, blk, :],
                                     start=False, stop=False)
                    nc.tensor.matmul(psX[:, 3:5], st, gt[:, bl, 128:130],
                                     start=False, stop=False)
                nc.tensor.matmul(psX[:, 0:5], iota_sb[:, :], zer_sb[:, 0:5],
                                 start=False, stop=True)
                rec = wpool.tile([128, 1], fp, tag="rec", name=f"rec{p}")
                nc.vector.tensor_scalar_max(rec[:], psX[:, 4:5], 1.0)
                nc.vector.reciprocal(rec[:], rec[:])
                nc.vector.tensor_scalar_mul(c4st[:, p, :], psX[:, 0:4], rec[:])
                fsum = wpool.tile([128, 128], fp, tag="fsum", name=f"fsum{p}")
                nc.vector.tensor_copy(fsum[:], psF[:])
                psT1 = ps128.tile([128, 128], fp, tag="b128", name=f"psT1{p}")
                nc.tensor.transpose(psT1[:], fsum[:], idfp_sb[:])
                mft = wpool.tile([128, 128], fp, tag="mft", name=f"mft{p}")
                nc.vector.tensor_scalar_mul(mft[:], psT1[:], rec[:])
                psT2 = ps128.tile([128, 128], fp, tag="b128", name=f"psT2{p}")
                nc.tensor.transpose(psT2[:], mft[:], idfp_sb[:])
                nc.vector.tensor_copy(meanF[:, p * 128:(p + 1) * 128], psT2[:])
                r0 = p * nblk_page * 128
                dst = pf_out[r0:r0 + nblk_page * 128, :].rearrange(
                    "(bl q) c -> q bl c", q=128)
                nc.sync.dma_start(dst, gt[:, :, 0:128])
            if "b" in PHASES:
                nc.sync.dma_start(
                    c4_out[:, :].rearrange("(p q) c -> q p c", q=128), c4st[:])

            # ================= phase C: cluster MLP ==========
            NSPLIT = [(0, 512), (512, 256)] if "c" in PHASES else []
            hC = ppool.tile([128, 2, CMAX], bf, tag="hC", name="hC")
            for mc in range(2 if "c" in PHASES else 0):
                for (n0, nw) in NSPLIT:
                    ps = ps512.tile([128, TILE], fp, tag="b512",
                                    name=f"psH{mc}_{n0}")
                    nc.tensor.matmul(ps[:, 0:nw],
                                     p1_sb[:, mc * 128:(mc + 1) * 128],
                                     meanF[:, n0:n0 + nw], start=True,
                                     stop=True)
                    nc.scalar.activation(hC[:, mc, n0:n0 + nw], ps[:, 0:nw],
                                         AF.Relu, bias=pb1_sb[:, mc:mc + 1])
            pstg = ppool.tile([128, NPAGES, 768], fp, tag="pstg", name="pstg")
            for mc in range(6 if "c" in PHASES else 0):
                prj = wpool.tile([128, CMAX], bf, tag="prj", name=f"prj{mc}")
                for (n0, nw) in NSPLIT:
                    ps = ps512.tile([128, TILE], fp, tag="b512",
                                    name=f"psP{mc}_{n0}")
                    for kc in range(2):
                        nc.tensor.matmul(ps[:, 0:nw],
                                         p2_sb[:, kc, mc * 128:(mc + 1) * 128],
                                         hC[:, kc, n0:n0 + nw],
                                         start=(kc == 0), stop=(kc == 1))
                    nc.scalar.activation(prj[:, n0:n0 + nw], ps[:, 0:nw],
                                         AF.Identity, bias=pb2_sb[:, mc:mc + 1])
                for cb in range(NPAGES):
                    psT = ps128.tile([128, 128], bf, tag="b128",
                                     name=f"psTP{mc}_{cb}")
                    nc.tensor.transpose(psT[:], prj[:, cb * 128:(cb + 1) * 128],
                                        idbf_sb[:])
                    nc.vector.tensor_copy(
                        pstg[:, cb, mc * 128:(mc + 1) * 128], psT[:])
            if "c" in PHASES:
                nc.sync.dma_start(
                    cp_out[:, :].rearrange("(cb q) c -> q cb c", q=128),
                    pstg[:])

    nc.compile()
    return nc


def _preprocess(inputs):
    import ml_dtypes
    bf = ml_dtypes.bfloat16

    inp = {k: np.ascontiguousarray(np.asarray(v)) for k, v in inputs.items()}
    points = inp["points"].astype(f32, copy=False)
    batch_idx = inp["batch_idx"].astype(np.int32, copy=False)
    bev = inp["bev"].astype(f32, copy=False)

    s1v = inp["g1"] / np.sqrt(inp["v1"] + BN_EPS)
    W1f = (inp["W1"] * s1v[None, :]).astype(f32)
    b1f = ((inp["b1"] - inp["m1"]) * s1v + inp["be1"]).astype(f32)
    s2v = inp["g2"] / np.sqrt(inp["v2"] + BN_EPS)
    S1f = (inp["S1"] * s2v[None, :]).astype(f32)
    sb1f = ((inp["sb1"] - inp["m2"]) * s2v + inp["be2"]).astype(f32)

    gx = np.clip(np.floor((points[:, 0] - X0) / VOX).astype(np.int32), 0, W - 1)
    gy = np.clip(np.floor((points[:, 1] - X0) / VOX).astype(np.int32), 0, H - 1)
    cid = np.floor((points[:, :2] - X0) / CLUSTER).astype(np.int32)
    cinds = cid[:, 0] * 10000 + cid[:, 1] * 100 + batch_idx
    uniq, inv = np.unique(cinds, return_inverse=True)
    inv = inv.astype(np.int64)
    n_real = len(uniq)

    cid_of_rank = uniq // 10000
    core_of_rank = np.searchsorted(BOUNDS, cid_of_rank, side="right") - 1
    rank_start = [int(np.searchsorted(core_of_rank, i))
                  for i in range(NCORES + 1)]
    pt_order = np.argsort(inv, kind="stable")
    inv_sorted = inv[pt_order]

    core_sel = []
    nblk_page = 1
    for i in range(NCORES):
        nclus = rank_start[i + 1] - rank_start[i]
        assert nclus <= CMAX, nclus
        lo = np.searchsorted(inv_sorted, rank_start[i], side="left")
        hi = np.searchsorted(inv_sorted, rank_start[i + 1], side="left")
        sel = pt_order[lo:hi]
        local_rank = inv[sel] - rank_start[i]
        page = local_rank // PAGE
        cnt = np.bincount(page, minlength=NPAGES)
        nblk_page = max(nblk_page, int(np.ceil(cnt.max() / 128)))
        core_sel.append((sel, local_rank, page))
    nblk = NPAGES * nblk_page
    nslots = nblk * 128

    eye_bf = np.eye(128).astype(bf)
    eye_fp = np.eye(128, dtype=f32)
    iota = np.broadcast_to(np.arange(PAGE, dtype=f32),
                           (128, PAGE)).astype(bf)
    w1_c = np.ascontiguousarray(
        W1f.reshape(3, 128, 256).transpose(1, 0, 2)).astype(bf)
    b1_c = np.ascontiguousarray(b1f.reshape(2, 128).T)
    w2_c = np.ascontiguousarray(
        inp["W2"].reshape(2, 128, 128).transpose(1, 0, 2)).astype(bf)
    b2_c = inp["b2"].reshape(128, 1).astype(f32)
    s1_c = np.zeros((128, 128), f32)
    s1_c[:, 0:64] = S1f
    s1_c = s1_c.astype(bf)
    sb1_c = np.zeros((128, 1), f32)
    sb1_c[0:64, 0] = sb1f
    s2_c = np.zeros((128, 1), f32)
    s2_c[0:64, 0] = inp["S2"][:, 0]
    s2_c = s2_c.astype(bf)
    sb2_c = np.full((128, 1), inp["sb2"][0], f32)
    p1_c = inp["P1"].astype(bf)
    pb1_c = np.ascontiguousarray(inp["pb1"].reshape(2, 128).T).astype(f32)
    p2_c = np.ascontiguousarray(
        inp["P2"].reshape(2, 128, 768).transpose(1, 0, 2)).astype(bf)
    pb2_c = np.ascontiguousarray(inp["pb2"].reshape(6, 128).T).astype(f32)

    in_maps = []
    core_meta = []
    core_cells = []
    bands = [[CELLS, 0] for _ in range(NPAGES)]
    for i in range(NCORES):
        a, bnd = BOUNDS[i], BOUNDS[i + 1]
        x0 = max(0, 5 * a - 1)
        x1 = min(W, 5 * bnd + 1)
        assert x1 - x0 <= W_SLAB
        slab = np.zeros((B, C, H, W_SLAB), f32)
        slab[:, :, :, :x1 - x0] = bev[:, :, :, x0:x1]
        # x-major: [xl, ck, p, b*256+y]
        bev_c = np.ascontiguousarray(
            slab.reshape(B, 3, 128, H, W_SLAB)
            .transpose(4, 1, 2, 0, 3)).reshape(NT, 3, 128, TILE).astype(bf)

        sel, local_rank, page = core_sel[i]
        slots = np.full(nslots, -1, np.int64)
        cval = np.full((nslots,), -1.0, f32)
        for p in range(NPAGES):
            pp = sel[page == p]
            base = p * nblk_page * 128
            slots[base:base + len(pp)] = pp
            cval[base:base + len(pp)] = (local_rank[page == p]
                                         - p * PAGE).astype(f32)
        valid = slots >= 0
        vp = slots[valid]
        cell = np.zeros(nslots, np.int64)
        lx = gx[vp] - x0
        assert lx.min() >= 0 and lx.max() < W_SLAB, (lx.min(), lx.max())
        cell[valid] = (lx * B + batch_idx[vp]) * H + gy[vp]
        for p in range(NPAGES):
            pc = cell[p * nblk_page * 128:(p + 1) * nblk_page * 128][
                valid[p * nblk_page * 128:(p + 1) * nblk_page * 128]]
            if len(pc):
                bands[p][0] = min(bands[p][0], int(pc.min()))
                bands[p][1] = max(bands[p][1], int(pc.max()) + 1)
        core_meta.append((slots, valid, vp))
        core_cells.append(cell)

        cval_c = np.ascontiguousarray(cval.reshape(nblk, 128).T).astype(bf)
        xyzp = np.zeros((nslots, 3), f32)
        xyzp[valid] = points[vp, 0:3]
        xyzp_c = np.ascontiguousarray(
            xyzp.reshape(nblk, 128, 3).transpose(1, 0, 2)).astype(bf)

        in_maps.append(dict(
            bev_in=bev_c, cval_in=cval_c, xyzp_in=xyzp_c,
            iota_in=iota, zer_in=np.zeros((128, 8), f32).astype(bf),
            idbf_in=eye_bf, idfp_in=eye_fp,
            w1_in=w1_c, b1_in=b1_c, w2_in=w2_c, b2_in=b2_c, s1_in=s1_c,
            sb1_in=sb1_c, s2_in=s2_c, sb2_in=sb2_c, p1_in=p1_c,
            pb1_in=pb1_c, p2_in=p2_c, pb2_in=pb2_c,
        ))

    for p in range(NPAGES):
        if bands[p][1] <= bands[p][0]:
            bands[p] = [0, 1]
    bands = [tuple(bd) for bd in bands]

    # gather indices relative to each page's band start
    for i in range(NCORES):
        cell = core_cells[i]
        gidx = np.zeros((128, nblk * 8), np.int16)
        for p in range(NPAGES):
            lo = bands[p][0]
            pg = np.maximum(cell[p * nblk_page * 128:(p + 1) * nblk_page * 128]
                            - lo, 0)
            assert pg.max() < bands[p][1] - lo
            pat = pg.reshape(-1, 16).T.astype(np.int16)
            gidx[:, p * nblk_page * 8:(p + 1) * nblk_page * 8] = np.tile(
                pat, (8, 1))
        in_maps[i]["gidx_in"] = gidx

    return in_maps, core_meta, core_cells, nblk_page, bands, rank_start, n_real


def _postprocess(inp, results, core_meta, core_cells, rank_start, n_real):
    point_feats = np.zeros((N, 128), f32)
    seg_logits = np.zeros((N, 1), f32)
    cluster_proj = np.zeros((MAXC, 768), f32)
    cluster_xyz = np.zeros((MAXC, 3), f32)
    cluster_seg = np.zeros((MAXC, 1), f32)

    for i in range(NCORES):
        r = results[i]
        slots, valid, vp = core_meta[i]
        cell = core_cells[i]
        point_feats[vp] = r["pf_out"][valid].astype(f32)
        gl = r["gl_out"]
        cl = cell[valid]
        seg_logits[vp, 0] = gl[cl % 128, cl // 128]
        g0, g1 = rank_start[i], rank_start[i + 1]
        nr = g1 - g0
        cluster_proj[g0:g1] = r["cp_out"][:nr]
        cluster_xyz[g0:g1] = r["c4_out"][:nr, 0:3]
        cluster_seg[g0:g1, 0] = r["c4_out"][:nr, 3]

    if n_real < MAXC:
        empty_row = (np.maximum(inp["pb1"].astype(np.float64), 0)
                     @ inp["P2"].astype(np.float64)
                     + inp["pb2"]).astype(f32)
        cluster_proj[n_real:] = empty_row[None, :]

    return point_feats, seg_logits, cluster_proj, cluster_xyz, cluster_seg


def kernel(**inputs):
    global LAST_RESULTS
    _install_ntff_shim()
    from concourse.bass_utils import run_bass_kernel_spmd

    inp = {k: np.ascontiguousarray(np.asarray(v)) for k, v in inputs.items()}
    (in_maps, core_meta, core_cells, nblk_page, bands, rank_start,
     n_real) = _preprocess(inp)
    nc = _build_program(nblk_page, bands)
    res = run_bass_kernel_spmd(
        nc, in_maps, core_ids=list(range(NCORES)), trace=TRACE,
        trace_cores=list(range(NCORES)) if TRACE else None)
    LAST_RESULTS = res
    return _postprocess(inp, res.results, core_meta, core_cells, rank_start,
                        n_real)
